# revision 1
# baseline (speedup 1.0000x reference)
"""Trainium2 Bass kernel for nn_CausalPropagationAdjacency.

Shapes (hardcoded): B=4, T=12, N=512, D=128, L=4, H=64.
Pipeline: lag encoders (Linear D->H, ReLU, Linear H->D, mean over L lags),
pairwise scorer sigmoid(relu(src_i+tgt_j+bs1)@Ws2+bs2), threshold 0.1, zero
diagonal, enhanced = A + 0.5 A^2 + 0.25 A^3, normalize by per-batch max.

Sharding: 8 cores = 4 batch-pairs. Core c: batch b=c//2, scores source rows
[half*256, half*256+256) (half=c%2). Adjacency slabs are AllGather'd within
the pair in TWO chunks (the first hides under scoring; a dummy warmup
AllGather at kernel start absorbs the first-collective setup cost). Each core
then computes the full enhanced matrix (hops are cheap) so no second
collective is needed for the global max. Host takes core 2b's output.

SPMD: one program for all cores; per-core behavior differs only through input
data (xlagT = batch lag slices, xsrcT = this core's half), both pre-transposed
to (D-partition, node-free) bf16 by the host.

Pairwise stage: per source i one fused DVE tensor_scalar (add + max0, bf16
out) or ACT Relu-with-bias produces relu(src_i+tgt+bs1) as a (128,512) bf16
tile; a matmul against a 64-wide sliding window of the packed weight buffer
(w2 embedded in one column) accumulates row i%64 of a (64,512) score block in
PSUM — the D-reduction runs at full PE streaming rate. Four score groups give
early sigmoid/threshold completion for the chunked collectives.

Precision: scoring path in bf16 (error ~4e-6 through the sigmoid); the
adjacency crosses the collective as a bf16 RESIDUAL (adj-0.5: values cluster
at 0.5 and exact zeros stay exact, so bf16 keeps ~fp32-level absolute
precision); hops (A^2, A^3) in fp32. End-to-end rel err ~3.9e-5.
"""

import sys
import types
import numpy as np
import ml_dtypes

import concourse.bacc as bacc
import concourse.bass as bass
import concourse.bass_isa as bass_isa
import concourse.mybir as mybir
import concourse.tile as tile
from concourse.bass_utils import run_bass_kernel_spmd

B, T, N, D = 4, 12, 512, 128
L, H = 4, 64
THRESH = 0.1
NCORES = 8
NHALF = N // 2
NT = N // 128
F32 = mybir.dt.float32
BF16 = mybir.dt.bfloat16
AF = mybir.ActivationFunctionType
ALU = mybir.AluOpType

# pairwise engine assignment per i%16 (DVE ~355ns/tile, ACT ~600ns/tile;
# GPSIMD is useless here: 7.6us/tile AND it stalls DVE via the shared port)
ACT_POS = {1, 4, 7, 10, 13}
GP_POS = set()
GP_CUTOFF = 0


def _build_nc():
    nc = bacc.Bacc("TRN2", target_bir_lowering=False, debug=False,
                   num_devices=NCORES)
    xlagT = nc.dram_tensor("xlagT", [L, D, N], BF16, kind="ExternalInput")
    xsrcT = nc.dram_tensor("xsrcT", [L, D, NHALF], BF16, kind="ExternalInput")
    # packed bf16 weights: [w1r(L*H=256) | ws1s(128) | ws1t(128) | zwin(255)
    #   | identity(128) | 0.5*identity(128) | bitcast f32 [bmean|bs1|bs2] (6)]
    wpk = nc.dram_tensor("wpk", [128, 2054], BF16, kind="ExternalInput")
    # w2r (64, L*D) bf16 + b1 (64, L) f32 bitcast to 2*L bf16 cols
    w2r = nc.dram_tensor("w2r", [H, L * D + 2 * L], BF16,
                         kind="ExternalInput")
    outfull = nc.dram_tensor("outfull", [N, N], F32, kind="ExternalOutput")

    with tile.TileContext(nc) as tc:
        _emit(nc, tc, xlagT, xsrcT, wpk, w2r, outfull)
    nc.compile()
    return nc


def _emit(nc, tc, xlagT, xsrcT, wpk, w2r, outfull):
    from contextlib import ExitStack
    ctx = ExitStack()
    with ctx:
        consts = ctx.enter_context(tc.tile_pool(name="consts", bufs=1))
        sb = ctx.enter_context(tc.tile_pool(name="sb", bufs=1))
        relup = ctx.enter_context(tc.tile_pool(name="relu", bufs=10))
        workp = ctx.enter_context(tc.tile_pool(name="work", bufs=4))
        psA = ctx.enter_context(tc.tile_pool(name="psA", bufs=2, space="PSUM"))
        psB = ctx.enter_context(tc.tile_pool(name="psB", bufs=2, space="PSUM"))
        psE = ctx.enter_context(tc.tile_pool(name="psE", bufs=4, space="PSUM"))
        dram = ctx.enter_context(tc.tile_pool(name="dram", bufs=1,
                                              space="DRAM"))

        # ---- input DMAs (few, big; xfull rides the gpsimd queue) ----
        xsrc = consts.tile([D, L, NHALF], BF16, tag="xs")
        nc.sync.dma_start(xsrc[:], xsrcT.ap().rearrange("l d n -> d l n"))
        wpks = consts.tile([128, 2054], BF16, tag="wpk")
        nc.sync.dma_start(wpks[:], wpk[:])
        w2pk = consts.tile([H, L * D + 2 * L], BF16, tag="w2")
        nc.sync.dma_start(w2pk[:], w2r[:])
        xfull = consts.tile([D, L, N], BF16, tag="xf")
        nc.sync.dma_start(xfull[:], xlagT.ap().rearrange("l d n -> d l n"))
        w2sb = w2pk[:, 0:L * D].rearrange("h (l d) -> h l d", l=L)
        b1sb = w2pk[:, L * D:L * D + 2 * L].bitcast(F32)
        w1sb = wpks[:, 0:256].rearrange("d (l h) -> d l h", l=L)
        ws1s_sb = wpks[:, 256:384]
        ws1t_sb = wpks[:, 384:512]
        zw = wpks[:, 512:767]
        idbf = wpks[:, 767:895]
        idhbf = wpks[:, 895:1023]
        fpks = wpks[:, 1024:1030].bitcast(F32)
        idf32 = wpks[:, 1030:1286].bitcast(F32)
        idh32 = wpks[:, 1286:1542].bitcast(F32)
        halfc = wpks[:, 1542:1546].bitcast(F32)  # [:,0]=+0.5, [:,1]=-0.5
        bmean_sb = fpks[:, 0:1]
        bs1_sb = fpks[:, 1:2]
        bs2_sb = fpks[:, 2:3]

        # ---- dummy warmup AllGather: absorbs first-collective setup ----
        warm_in = dram.tile([1, 2], BF16, tag="warmi", name="warm_in")
        warm_out = dram.tile([2, 2], BF16, tag="warmo", name="warm_out")
        nc.gpsimd.dma_start(warm_in[:], wpk[0:1, 0:2])
        nc.gpsimd.collective_compute(
            "AllGather", ALU.bypass,
            replica_groups=[[0, 1], [2, 3], [4, 5], [6, 7]],
            ins=[warm_in.opt()],
            outs=[warm_out.opt()],
        )

        # ---- encoders: (D-part, node) bf16 in, agg out ----
        def encoder(xt, n_nodes, tag):
            encT = psB.tile([D, n_nodes], F32, tag="acc")
            for l in range(L):
                hT = psA.tile([H, n_nodes], F32, tag="t")
                nc.tensor.matmul(hT[:], w1sb[:, l, :], xt[:, l, :],
                                 start=True, stop=True)
                hsb = workp.tile([H, n_nodes], BF16, tag=f"h{tag}")
                nc.scalar.activation(hsb[:], hT[:], AF.Relu,
                                     bias=b1sb[:, l:l + 1], scale=1.0)
                nc.tensor.matmul(encT[:], w2sb[:, l, :], hsb[:],
                                 start=(l == 0), stop=(l == L - 1))
            agg_bf = sb.tile([D, n_nodes], BF16, tag=f"agg{tag}")
            nc.scalar.activation(agg_bf[:], encT[:], AF.Identity,
                                 bias=bmean_sb, scale=1.0 / L)
            return agg_bf

        agg_s = encoder(xsrc, NHALF, "s")
        agg_f = encoder(xfull, N, "f")

        # ---- projections ----
        src_ps = psA.tile([D, NHALF], F32, tag="t")
        nc.tensor.matmul(src_ps[:], ws1s_sb, agg_s[:], start=True,
                         stop=True)
        srcT = sb.tile([D, NHALF], F32, tag="srcf")
        nc.scalar.activation(srcT[:], src_ps[:], AF.Identity,
                             bias=bs1_sb, scale=1.0)
        tgt_ps = psA.tile([D, N], F32, tag="t")
        nc.tensor.matmul(tgt_ps[:], ws1t_sb, agg_f[:], start=True,
                         stop=True)
        tgtT_bf = sb.tile([D, N], BF16, tag="tgtbf")
        nc.vector.tensor_copy(tgtT_bf[:], tgt_ps[:])

        # ---- SBUF homes for gathered adjacency and its transpose ----
        A = [sb.tile([128, N], F32, tag=f"A{kt}", name=f"A{kt}") for kt in range(NT)]
        AT = [sb.tile([128, N], F32, tag=f"AT{kt}", name=f"AT{kt}") for kt in range(NT)]
        a2ps = {}
        a2sb = [sb.tile([128, N], F32, tag=f"a2{it}", name=f"a2sb{it}") for it in range(NT)]
        bounce = [dram.tile([128, N], BF16, tag=f"bnc{c}", name=f"bnc{c}")
                  for c in range(2)]
        full = [dram.tile([256, N], BF16, tag=f"full{c}", name=f"full{c}")
                for c in range(2)]

        # gathered chunk c holds global k-tiles {c, c+2}
        def load_chunk(c):
            nc.gpsimd.collective_compute(
                "AllGather", ALU.bypass,
                replica_groups=[[0, 1], [2, 3], [4, 5], [6, 7]],
                ins=[bounce[c].opt()],
                outs=[full[c].opt()],
            )
            for piece, kt in enumerate((c, c + 2)):
                rsb = workp.tile([128, N], BF16, tag="rsb",
                                 name=f"rsb{c}_{piece}")
                nc.sync.dma_start(
                    rsb[:], full[c][piece * 128:(piece + 1) * 128, :])
                nc.scalar.activation(A[kt][:], rsb[:], AF.Identity,
                                     bias=halfc[:, 0:1], scale=1.0)
                nc.gpsimd.affine_select(
                    A[kt][:], A[kt][:], pattern=[[1, N]],
                    compare_op=ALU.not_equal, fill=0.0,
                    base=-(128 * kt), channel_multiplier=-1)

        def transpose_of(it, kt, use_act):
            """AT[kt][:, it-block] = (A[it][:, kt-block])^T."""
            pool = psA if (it + kt) % 2 == 0 else psB
            tp = pool.tile([128, 128], F32, tag="t" if pool is psA
                           else "acc", name=f"tp{it}_{kt}")
            nc.tensor.transpose(tp[:], A[it][:, kt * 128:(kt + 1) * 128],
                                idf32)
            dst = AT[kt][:, it * 128:(it + 1) * 128]
            if use_act:
                nc.scalar.copy(dst, tp[:])
            else:
                nc.vector.tensor_copy(dst, tp[:])

        def a2_step(it, kt, start, stop):
            if it not in a2ps:
                a2ps[it] = psE.tile([128, N], F32, tag="E",
                                    name=f"a2ps{it}")
            nc.tensor.matmul(a2ps[it][:], AT[kt][:, it * 128:(it + 1) * 128],
                             A[kt][:], start=start, stop=stop)

        # early tail work, sprinkled into the 2nd scoring block:
        # transposes sourced from A[0]/A[2], then a2 partial K-steps {0,2}
        early = []
        for it in (0, 2):
            for kt in range(NT):
                early.append(
                    lambda it=it, kt=kt: transpose_of(it, kt, (it + kt) % 2))
        for it in (0, 2):
            for kt in (0, 2):
                early.append(
                    lambda it=it, kt=kt: a2_step(it, kt, kt == 0, False))

        # ---- pairwise scoring: 4 groups of 64 source rows (M=64) ----
        # w2 sits at wpk column 639; window [639-p : 703-p] puts it in
        # column p of a 64-wide lhsT -> psum row p of the (64,512) group
        for g in range(4):
            score_ps = psB.tile([64, N], F32, tag="acc", name=f"scps{g}")
            for p in range(64):
                i = g * 64 + p
                rt = relup.tile([D, N], BF16, tag="rt")
                act_pos = ACT_POS | ({9} if i < 128 else set())
                if i % 16 in act_pos:
                    nc.scalar.activation(rt[:], tgtT_bf[:], AF.Relu,
                                         bias=srcT[:, i:i + 1], scale=1.0)
                else:
                    nc.vector.tensor_scalar(rt[:], tgtT_bf[:],
                                            srcT[:, i:i + 1], 0.0,
                                            ALU.add, ALU.max)
                nc.tensor.matmul(score_ps[:], wpks[:, 639 - p:703 - p],
                                 rt[:], start=(p == 0), stop=(p == 63))
            score_sb = workp.tile([64, N], F32, tag="score",
                                  name=f"scsb{g}")
            nc.scalar.activation(score_sb[:], score_ps[:], AF.Sigmoid,
                                 bias=bs2_sb[0:64, :], scale=1.0)
            adjs = workp.tile([64, N], F32, tag="adjs", name=f"adj{g}")
            nc.vector.scalar_tensor_tensor(adjs[:], score_sb[:], THRESH,
                                           score_sb[:], ALU.is_gt, ALU.mult)
            # residual encode: adj values cluster near 0.5 (and exact 0);
            # adj-0.5 in bf16 keeps ~fp32-level absolute precision here
            resid = workp.tile([64, N], BF16, tag="resid", name=f"rs{g}")
            nc.scalar.activation(resid[:], adjs[:], AF.Identity,
                                 bias=halfc[0:64, 1:2], scale=1.0)
            nc.sync.dma_start(bounce[g // 2][(g % 2) * 64:(g % 2) * 64 + 64, :],
                              resid[:])
            if g % 2 == 1:
                load_chunk(g // 2)
        # early-tail ops (A[0]/A[2] transposes + partial a2): run in the
        # collective-#2 window; gated only on chunk 0, which is long done
        while early:
            early.pop(0)()
        # ---- late tail, emitted in dependency-readiness order ----
        # a2[0], a2[2] late K-steps only need A[1]/A[3] as rhs (their AT
        # slices came from A[0]/A[2], transposed early) -> finish + evac first
        for it in (0, 2):
            a2_step(it, 1, False, False)
            a2_step(it, 3, False, True)
            nc.vector.tensor_copy(a2sb[it][:], a2ps[it][:])
        # transposes sourced from A[1]/A[3], then a2[1], a2[3]
        for it in (1, 3):
            for kt in range(NT):
                transpose_of(it, kt, (it + kt) % 2 == 0)
        for it in (1, 3):
            for kt in range(NT):
                a2_step(it, kt, kt == 0, kt == 3)
            nc.vector.tensor_copy(a2sb[it][:], a2ps[it][:])

        # ---- E = A@a2 + 0.5*a2 + A, accumulated in PSUM ----
        E = []
        mx4 = sb.tile([128, NT], F32, tag="mx4")
        for it in range(NT):
            e_ps = psE.tile([128, N], F32, tag="E")
            nc.tensor.matmul(e_ps[:], idf32, A[it][:], start=True, stop=False)
            nc.tensor.matmul(e_ps[:], idh32, a2sb[it][:], start=False,
                             stop=False)
            for kt in range(NT):
                nc.tensor.matmul(e_ps[:], AT[kt][:, it * 128:(it + 1) * 128],
                                 a2sb[kt][:], start=False, stop=(kt == 3))
            nc.vector.reduce_max(mx4[:, it:it + 1], e_ps[:],
                                 axis=mybir.AxisListType.X)
            E.append(e_ps)

        # ---- global max + normalize + write out ----
        mxp = sb.tile([128, 1], F32, tag="mxp")
        nc.vector.reduce_max(mxp[:], mx4[:], axis=mybir.AxisListType.X)
        mxall = sb.tile([128, 1], F32, tag="mxall")
        nc.gpsimd.partition_all_reduce(mxall[:], mxp[:], 128,
                                       bass_isa.ReduceOp.max)
        denom = sb.tile([128, 1], F32, tag="denom")
        nc.vector.tensor_scalar(denom[:], mxall[:], 1e-8, None, ALU.add)
        recip = sb.tile([128, 1], F32, tag="recip")
        nc.vector.reciprocal(recip[:], denom[:])
        for it in range(NT):
            ot = workp.tile([128, N], F32, tag="ot")
            if it % 2 == 0:
                nc.vector.tensor_scalar(ot[:], E[it][:], recip[:, 0:1], None,
                                        ALU.mult)
            else:
                nc.scalar.mul(ot[:], E[it][:], recip[:, 0:1])
            nc.sync.dma_start(outfull[it * 128:(it + 1) * 128, :], ot[:])


_NC_CACHE = {}


def _get_nc():
    if "nc" not in _NC_CACHE:
        _NC_CACHE["nc"] = _build_nc()
    return _NC_CACHE["nc"]


def _install_ntff_hook():
    try:
        from antenv.axon_hooks import get_axon_ntff_profile_hook  # noqa: F401
        return
    except ImportError:
        pass
    try:
        import importlib.util
        spec = importlib.util.spec_from_file_location(
            "trn_boot_mod", "/root/.axon_site/trn_agent_boot/trn_boot.py")
        tb = importlib.util.module_from_spec(spec)
        spec.loader.exec_module(tb)
        hook = tb._ntff_profile_via_ctypes("/opt/axon/libaxon_pjrt.so")
        m = types.ModuleType("antenv.axon_hooks")
        m.get_axon_ntff_profile_hook = lambda: hook
        m.set_axon_ntff_profile_hook = lambda h: None
        sys.modules["antenv.axon_hooks"] = m
    except Exception:
        pass


def _bf(a):
    return np.ascontiguousarray(a).astype(ml_dtypes.bfloat16)


def _prep_in_maps(x, W1, b1, W2, b2, Ws1, bs1, Ws2, bs2):
    x = np.asarray(x, np.float32)
    W1 = np.asarray(W1, np.float32)
    b1 = np.asarray(b1, np.float32)
    W2 = np.asarray(W2, np.float32)
    b2 = np.asarray(b2, np.float32)
    Ws1 = np.asarray(Ws1, np.float32)
    bs1 = np.asarray(bs1, np.float32)
    Ws2 = np.asarray(Ws2, np.float32)
    bs2 = np.asarray(bs2, np.float32)

    Tdim = x.shape[1]
    lag_idx = [max(0, Tdim - 1 - l) for l in range(L)]
    xl = x[:, lag_idx]                            # (B, L, N, D)
    xlT = np.swapaxes(xl, 2, 3)                   # (B, L, D, N)

    zwin = np.zeros((128, 255), np.float32)
    zwin[:, 127] = Ws2[:, 0]
    fpk = np.stack([b2.mean(axis=0), bs1,
                    np.full(128, bs2[0], np.float32)], axis=1)
    fpk_bf = np.ascontiguousarray(fpk.astype(np.float32)).view(
        ml_dtypes.bfloat16)                               # (128, 6)
    wpk = np.concatenate([
        _bf(np.transpose(W1, (1, 0, 2)).reshape(D, L * H)),
        _bf(Ws1[:D]),
        _bf(Ws1[D:]),
        _bf(zwin),
        _bf(np.eye(128, dtype=np.float32)),
        _bf(0.5 * np.eye(128, dtype=np.float32)),
        np.zeros((128, 1), ml_dtypes.bfloat16),           # pad to even col
        fpk_bf,
        np.eye(128, dtype=np.float32).view(ml_dtypes.bfloat16),
        (0.5 * np.eye(128, dtype=np.float32)).view(ml_dtypes.bfloat16),
        np.concatenate([np.full((128, 1), 0.5, np.float32),
                        np.full((128, 1), -0.5, np.float32)],
                       axis=1).view(ml_dtypes.bfloat16),
        np.zeros((128, 2054 - 1546), ml_dtypes.bfloat16),
    ], axis=1)                                            # (128, 2054)
    b1_bf = np.ascontiguousarray(b1.T.astype(np.float32)).view(
        ml_dtypes.bfloat16)                               # (64, 2L)
    w2pk = np.concatenate(
        [_bf(np.transpose(W2, (1, 0, 2)).reshape(H, L * D)), b1_bf], axis=1)

    common = {
        "wpk": np.ascontiguousarray(wpk),
        "w2r": np.ascontiguousarray(w2pk),
    }
    in_maps = []
    for c in range(NCORES):
        b, half = c // 2, c % 2
        m = dict(common)
        m["xlagT"] = _bf(xlT[b])
        m["xsrcT"] = _bf(xlT[b][:, :, half * NHALF:(half + 1) * NHALF])
        in_maps.append(m)
    return in_maps


def _run(inputs, trace=False):
    nc = _get_nc()
    in_maps = _prep_in_maps(**inputs)
    if trace:
        _install_ntff_hook()
    res = run_bass_kernel_spmd(nc, in_maps, core_ids=list(range(NCORES)),
                               trace=trace)
    out = np.stack([res.results[2 * b]["outfull"] for b in range(B)], axis=0)
    return out, res


def kernel(**inputs):
    out, _ = _run(inputs, trace=False)
    return out



# revision 28
# speedup vs baseline: 1.2106x; 1.2106x over previous
"""Trainium2 Bass kernel for nn_CausalPropagationAdjacency (v2).

Shapes (hardcoded): B=4, T=12, N=512, D=128, L=4, H=64.
Pipeline: lag encoders (Linear D->H, ReLU, Linear H->D, mean over L lags),
pairwise scorer sigmoid(relu(src_i+tgt_j+bs1)@Ws2+bs2), threshold 0.1, zero
diagonal, enhanced = A + 0.5 A^2 + 0.25 A^3, normalize by per-batch max.

Sharding: 8 cores = 4 batch-pairs. Core c: batch b=c//2, scores source rows
[256*half, 256*half+256) (half=c%2). Scoring runs in 4 groups of 64 rows; after
each group the tanh residual t (score = 0.5 + 0.5*t) is AllGather'd within the
pair as a 64-row chunk. Chunk c forms a complete 128-row block of the adjacency
in a PERMUTED row basis (global rows {64c..64c+64} u {256+64c..}), so its
transposes and partial A@A accumulation steps run while later groups still
score - only chunk 3's work sits on the tail. Each core computes the full
enhanced matrix; host takes core 2b's output.

Speed tricks vs v1: lag-pair fp8 DoubleRow encoder matmuls; scoring rows split
DVE-bf16 (sliding zwin window) / ACT-fp8 pairs (DoubleRow window, 2 rows per
matmul); all hop/E matmuls in float32r (1 cyc/row at 512 free vs 4 for fp32);
global max via transpose+broadcast matmuls instead of partition_all_reduce;
fp8/scale-64 numerics (Ws1,w2,W1 scaled by 64 so fp8 values are normal-range;
1/4096 folded into the tanh activation scale).

Precision: scoring bf16/fp8 paths agree to ~1e-4 relative on |score-0.5|;
adjacency crosses the collective as the bf16 tanh residual; hops in f32 data
with f32r matmuls. E uses exact reference ratios (a2sb = 0.25*a2, idh = 2*I).
"""

import sys
import types
import numpy as np
import ml_dtypes

import concourse.bacc as bacc
import concourse.bass as bass
import concourse.bass_isa as bass_isa
import concourse.mybir as mybir
import concourse.tile as tile
from concourse.bass_utils import run_bass_kernel_spmd

B, T, N, D = 4, 12, 512, 128
L, H = 4, 64
THRESH = 0.1
NCORES = 8
NHALF = N // 2
NT = N // 128
SCL = 64.0
F32 = mybir.dt.float32
F32R = mybir.dt.float32r
BF16 = mybir.dt.bfloat16
FP8 = mybir.dt.float8e4
AF = mybir.ActivationFunctionType
ALU = mybir.AluOpType
DR = mybir.MatmulPerfMode.DoubleRow

# scoring row assignment within each 16: ACT fp8 pairs at 4,12; ACT bf16
# single at 9 (+14 on odd 16-blocks); DVE bf16 elsewhere
ACT_PAIR_POS = (4, 12)


def _act_single(p):
    p16, blk = p % 16, p // 16
    return p16 == 9 or (p16 == 14 and blk % 2 == 1)


def _build_nc():
    nc = bacc.Bacc("TRN2", target_bir_lowering=False, debug=False,
                   num_devices=NCORES)
    xlag = nc.dram_tensor("xlag", [D, L, N], FP8, kind="ExternalInput")
    xsrc = nc.dram_tensor("xsrc", [D, L, NHALF], FP8, kind="ExternalInput")
    wbf = nc.dram_tensor("wbf", [128, 1552], BF16, kind="ExternalInput")
    w1p8 = nc.dram_tensor("w1p8", [128, 2, 256], FP8, kind="ExternalInput")
    w8 = nc.dram_tensor("w8", [128, 2, 128], FP8, kind="ExternalInput")
    outfull = nc.dram_tensor("outfull", [N, N], F32, kind="ExternalOutput")

    with tile.TileContext(nc) as tc:
        _emit(nc, tc, xlag, xsrc, wbf, w1p8, w8, outfull)
    nc.compile()
    return nc


def _emit(nc, tc, xlag, xsrc, wbf, w1p8, w8, outfull):
    from contextlib import ExitStack
    ctx = ExitStack()
    with ctx:
        consts = ctx.enter_context(tc.tile_pool(name="consts", bufs=1))
        sb = ctx.enter_context(tc.tile_pool(name="sb", bufs=1))
        relup = ctx.enter_context(tc.tile_pool(name="relu", bufs=8))
        rt8p = ctx.enter_context(tc.tile_pool(name="rt8", bufs=4))
        workp = ctx.enter_context(tc.tile_pool(name="work", bufs=4))
        # PSUM budget (8 banks): psh 1 + psacc 2 + psbig 4 (+1 spare)
        ps_h = ctx.enter_context(tc.tile_pool(name="psh", bufs=1, space="PSUM"))
        ps_acc = ctx.enter_context(tc.tile_pool(name="psacc", bufs=2,
                                                space="PSUM"))
        ps_big = ctx.enter_context(tc.tile_pool(name="psbig", bufs=4,
                                                space="PSUM"))
        dram = ctx.enter_context(tc.tile_pool(name="dram", bufs=1,
                                              space="DRAM"))

        # ---- input DMAs ----
        xlg = consts.tile([D, L, N], FP8, tag="xlg")
        nc.gpsimd.dma_start(xlg[:], xlag[:])
        xsr = consts.tile([D, L, NHALF], FP8, tag="xsr")
        nc.sync.dma_start(xsr[:], xsrc[:])
        wbfs = consts.tile([128, 1552], BF16, tag="wbf")
        nc.sync.dma_start(wbfs[:], wbf[:])
        w1s = consts.tile([128, 2, 256], FP8, tag="w1s")
        nc.sync.dma_start(w1s[:], w1p8[:])
        w8s = consts.tile([128, 2, 128], FP8, tag="w8s")
        nc.sync.dma_start(w8s[:], w8[:])

        w2stk = wbfs[:, 0:256]
        ws1s = wbfs[:, 256:384]
        ws1t = wbfs[:, 384:512]
        zwin = wbfs[:, 512:767]
        f32sec = wbfs[:, 768:1552].bitcast(F32)   # (128, 392) f32
        id1 = f32sec[:, 0:128]
        idh2 = f32sec[:, 128:256]
        onesrow = f32sec[:, 256:384]
        b1c = f32sec[:, 384:386]
        aggb = f32sec[:, 386:387]
        bs1c = f32sec[:, 387:388]
        bs2h = f32sec[:, 388:389]
        zcol = f32sec[:, 389:390]

        # f32r-typed copies of the identity blocks (walrus requires fp32r
        # matmul operands to come from fp32r-rounded producers, and DMA
        # doesn't count)
        idr = sb.tile([128, 256], F32, tag="idr")
        nc.vector.tensor_copy(idr[:, 0:128].bitcast(F32R), id1)
        nc.vector.tensor_copy(idr[:, 128:256].bitcast(F32R), idh2)
        id1r = idr[:, 0:128].bitcast(F32R)
        idh2r = idr[:, 128:256].bitcast(F32R)

        # ---- dummy warmup AllGather: absorbs first-collective setup ----
        warm_in = dram.tile([1, 2], BF16, tag="warmi", name="warm_in")
        warm_out = dram.tile([2, 2], BF16, tag="warmo", name="warm_out")
        nc.gpsimd.dma_start(warm_in[:], wbf[0:1, 0:2])
        nc.gpsimd.collective_compute(
            "AllGather", ALU.bypass,
            replica_groups=[[0, 1], [2, 3], [4, 5], [6, 7]],
            ins=[warm_in.opt()],
            outs=[warm_out.opt()],
        )

        # ---- encoders: fp8 lag-pair DoubleRow matmuls ----
        def encoder(xt, nn, tag):
            encp = ps_acc.tile([D, nn], F32, tag="acc", name=f"enc{tag}")
            for pair in range(2):
                h2 = ps_h.tile([128, nn], F32, tag="h", name=f"h2{tag}{pair}")
                nc.tensor.matmul(h2[:], w1s[:, :, 128 * pair:128 * pair + 128],
                                 xt[:, 2 * pair:2 * pair + 2, :],
                                 start=True, stop=True, perf_mode=DR)
                hsb = workp.tile([128, nn], BF16, tag="hsb",
                                 name=f"hsb{tag}{pair}")
                if pair == 0:
                    nc.vector.tensor_scalar(hsb[:], h2[:],
                                            b1c[:, pair:pair + 1], 0.0,
                                            ALU.add, ALU.max)
                else:
                    nc.scalar.activation(hsb[:], h2[:], AF.Relu,
                                         bias=b1c[:, pair:pair + 1], scale=1.0)
                nc.tensor.matmul(encp[:],
                                 w2stk[:, 128 * pair:128 * pair + 128],
                                 hsb[:], start=(pair == 0), stop=(pair == 1))
            agg = sb.tile([D, nn], BF16, tag=f"agg{tag}")
            nc.scalar.activation(agg[:], encp[:], AF.Identity, bias=aggb,
                                 scale=1.0)
            return agg

        agg_s = encoder(xsr, NHALF, "s")
        agg_f = encoder(xlg, N, "f")

        # ---- projections (64*Ws1 folded in; bs1*64 bias on src side) ----
        srcp = ps_acc.tile([D, NHALF], F32, tag="acc", name="srcp")
        nc.tensor.matmul(srcp[:], ws1s, agg_s[:], start=True, stop=True)
        srcT = sb.tile([D, NHALF], F32, tag="srcT")
        nc.scalar.activation(srcT[:], srcp[:], AF.Identity, bias=bs1c,
                             scale=1.0)
        tgtp = ps_acc.tile([D, N], F32, tag="acc", name="tgtp")
        nc.tensor.matmul(tgtp[:], ws1t, agg_f[:], start=True, stop=True)
        tgtT = sb.tile([D, N], BF16, tag="tgtT")
        nc.vector.tensor_copy(tgtT[:], tgtp[:])

        # ---- SBUF homes for gathered adjacency (permuted block basis) ----
        # block c rows (in order) = global rows 64c..64c+64, 256+64c..256+64c+64
        A = [sb.tile([128, N], F32, tag=f"A{c}", name=f"A{c}")
             for c in range(NT)]
        AT = [sb.tile([128, N], F32, tag=f"AT{c}", name=f"AT{c}")
              for c in range(NT)]
        a2sb = [sb.tile([128, N], F32, tag=f"a2{c}", name=f"a2sb{c}")
                for c in range(NT)]
        a2ps = {}
        bounce = [dram.tile([64, N], BF16, tag=f"bnc{c}", name=f"bnc{c}")
                  for c in range(NT)]
        full = [dram.tile([128, N], BF16, tag=f"full{c}", name=f"full{c}")
                for c in range(NT)]

        def chunk_load(c):
            tsb = workp.tile([128, N], BF16, tag="tsb", name=f"tsb{c}")
            nc.sync.dma_start(tsb[:], full[c][:])
            apre = workp.tile([128, N], F32, tag="apre", name=f"apre{c}")
            nc.vector.tensor_scalar(apre[:], tsb[:], 0.5, 0.5,
                                    ALU.mult, ALU.add)
            nc.vector.scalar_tensor_tensor(A[c][:].bitcast(F32R), apre[:],
                                           THRESH, apre[:],
                                           ALU.is_gt, ALU.mult)
            nc.gpsimd.affine_select(
                A[c][:].bitcast(F32R), A[c][:].bitcast(F32R),
                pattern=[[1, N]],
                compare_op=ALU.not_equal, fill=0.0,
                base=-(128 * c), channel_multiplier=-1)

        def chunk_transposes(c):
            """AT[kt][:, c-block] = (A[c][:, kt-block])^T for all kt."""
            for kt in range(NT):
                tp = ps_acc.tile([128, 128], F32R, tag="acc",
                                 name=f"tp{c}_{kt}")
                nc.tensor.transpose(
                    tp[:], A[c][:, kt * 128:(kt + 1) * 128].bitcast(F32R),
                    id1r)
                dst = AT[kt][:, c * 128:(c + 1) * 128].bitcast(F32R)
                if (c + kt) % 2 == 0:
                    nc.scalar.copy(dst, tp[:])
                else:
                    nc.vector.tensor_copy(dst, tp[:])

        def a2_step(it, kt, stop=False):
            if it not in a2ps:
                a2ps[it] = ps_big.tile([128, N], F32, tag="E",
                                       name=f"a2ps{it}")
            nc.tensor.matmul(a2ps[it][:],
                             AT[kt][:, it * 128:(it + 1) * 128].bitcast(F32R),
                             A[kt][:].bitcast(F32R),
                             start=(kt == 0), stop=stop)

        def a2_steps_for(c):
            for kt in range(c + 1):
                a2_step(c, kt)
            for it in range(c):
                a2_step(it, c)

        # ---- pairwise scoring: 4 groups of 64 source rows ----
        def score_rows(g, p_lo, p_hi, score_ps):
            p = p_lo
            while p < p_hi:
                i = 64 * g + p
                if p % 16 in ACT_PAIR_POS:
                    rt8 = rt8p.tile([D, 2, N], FP8, tag="rt8",
                                    name=f"rt8_{g}_{p}")
                    for k in range(2):
                        nc.scalar.activation(rt8[:, k, :], tgtT[:], AF.Relu,
                                             bias=srcT[:, i + k:i + k + 1],
                                             scale=1.0)
                    nc.tensor.matmul(score_ps[:], w8s[:, :, 62 - p:126 - p],
                                     rt8[:], start=(p == 0), stop=False,
                                     perf_mode=DR)
                    p += 2
                else:
                    rtb = relup.tile([D, N], BF16, tag="rtb",
                                     name=f"rtb_{g}_{p}")
                    if _act_single(p):
                        nc.scalar.activation(rtb[:], tgtT[:], AF.Relu,
                                             bias=srcT[:, i:i + 1], scale=1.0)
                    else:
                        nc.vector.tensor_scalar(rtb[:], tgtT[:],
                                                srcT[:, i:i + 1], 0.0,
                                                ALU.add, ALU.max)
                    nc.tensor.matmul(score_ps[:], zwin[:, 127 - p:191 - p],
                                     rtb[:], start=(p == 0), stop=(p == 63))
                    p += 1

        for g in range(4):
            score_ps = ps_acc.tile([64, N], F32, tag="acc", name=f"scps{g}")
            score_rows(g, 0, 16, score_ps)
            if g >= 1:
                chunk_load(g - 1)
            score_rows(g, 16, 64, score_ps)
            t_sb = workp.tile([64, N], BF16, tag="t_sb", name=f"t_sb{g}")
            nc.scalar.activation(t_sb[:], score_ps[:], AF.Tanh,
                                 bias=bs2h[0:64, :], scale=0.5 / 4096.0)
            nc.sync.dma_start(bounce[g][:], t_sb[:])
            nc.gpsimd.collective_compute(
                "AllGather", ALU.bypass,
                replica_groups=[[0, 1], [2, 3], [4, 5], [6, 7]],
                ins=[bounce[g].opt()],
                outs=[full[g].opt()],
            )
            if g >= 1:
                chunk_transposes(g - 1)
            if g >= 2:
                a2_steps_for(g - 2)

        # ---- tail: chunk 2 deferred steps, chunk 3, a2 finish ----
        a2_steps_for(2)
        chunk_load(3)
        chunk_transposes(3)
        for kt in range(3):
            a2_step(3, kt)
        for it in range(NT):
            a2_step(it, 3, stop=True)
            if it % 2 == 0:
                nc.vector.tensor_scalar(a2sb[it][:].bitcast(F32R),
                                        a2ps[it][:], 0.25, None, ALU.mult)
            else:
                nc.scalar.activation(a2sb[it][:].bitcast(F32R), a2ps[it][:],
                                     AF.Identity, bias=zcol, scale=0.25)

        # ---- E = A + 2*(0.25 a2) + AT@(0.25 a2) = A + 0.5 a2 + 0.25 a3 ----
        E = []
        mx4 = sb.tile([128, NT], F32, tag="mx4")
        for it in range(NT):
            e_ps = ps_big.tile([128, N], F32, tag="E", name=f"eps{it}")
            nc.tensor.matmul(e_ps[:], id1r, A[it][:].bitcast(F32R),
                             start=True, stop=False)
            nc.tensor.matmul(e_ps[:], idh2r, a2sb[it][:].bitcast(F32R),
                             start=False, stop=False)
            for kt in range(NT):
                nc.tensor.matmul(
                    e_ps[:],
                    AT[kt][:, it * 128:(it + 1) * 128].bitcast(F32R),
                    a2sb[kt][:].bitcast(F32R), start=False, stop=(kt == 3))
            nc.vector.reduce_max(mx4[:, it:it + 1], e_ps[:],
                                 axis=mybir.AxisListType.X)
            E.append(e_ps)

        # ---- global max via transpose+broadcast matmuls ----
        mxp = sb.tile([128, 1], F32, tag="mxp")
        nc.vector.reduce_max(mxp[:], mx4[:], axis=mybir.AxisListType.X)
        tp1 = ps_acc.tile([1, 128], F32, tag="acc", name="tp1")
        nc.tensor.matmul(tp1[:], mxp[:], id1, start=True, stop=True)
        mxrow = sb.tile([1, 128], F32, tag="mxrow")
        nc.vector.tensor_copy(mxrow[:], tp1[:])
        mx1 = sb.tile([1, 1], F32, tag="mx1")
        nc.vector.reduce_max(mx1[:], mxrow[:], axis=mybir.AxisListType.X)
        den = sb.tile([1, 1], F32, tag="den")
        nc.vector.tensor_scalar(den[:], mx1[:], 1e-8, None, ALU.add)
        rcp = sb.tile([1, 1], F32, tag="rcp")
        nc.vector.reciprocal(rcp[:], den[:])
        rb_ps = ps_acc.tile([128, 1], F32, tag="acc", name="rb_ps")
        nc.tensor.matmul(rb_ps[:], onesrow[0:1, :], rcp[:], start=True,
                         stop=True)
        rcol = sb.tile([128, 1], F32, tag="rcol")
        nc.vector.tensor_copy(rcol[:], rb_ps[:])

        # ---- normalize + write out (still in pi basis; host un-permutes) ----
        for it in range(NT):
            ot = workp.tile([128, N], F32, tag="ot", name=f"ot{it}")
            if it % 2 == 0:
                nc.vector.tensor_scalar(ot[:], E[it][:], rcol[:, 0:1], None,
                                        ALU.mult)
            else:
                nc.scalar.mul(ot[:], E[it][:], rcol[:, 0:1])
            nc.sync.dma_start(outfull[128 * it:128 * it + 128, :], ot[:])


_NC_CACHE = {}


def _get_nc():
    if "nc" not in _NC_CACHE:
        _NC_CACHE["nc"] = _build_nc()
    return _NC_CACHE["nc"]


def _install_ntff_hook():
    try:
        from antenv.axon_hooks import get_axon_ntff_profile_hook  # noqa: F401
        return
    except ImportError:
        pass
    try:
        import importlib.util
        spec = importlib.util.spec_from_file_location(
            "trn_boot_mod", "/root/.axon_site/trn_agent_boot/trn_boot.py")
        tb = importlib.util.module_from_spec(spec)
        spec.loader.exec_module(tb)
        hook = tb._ntff_profile_via_ctypes("/opt/axon/libaxon_pjrt.so")
        m = types.ModuleType("antenv.axon_hooks")
        m.get_axon_ntff_profile_hook = lambda: hook
        m.set_axon_ntff_profile_hook = lambda h: None
        sys.modules["antenv.axon_hooks"] = m
    except Exception:
        pass


def _f8(a):
    return np.ascontiguousarray(a).astype(ml_dtypes.float8_e4m3fn)


def _bf(a):
    return np.ascontiguousarray(a).astype(ml_dtypes.bfloat16)


def _f32bf(a):
    """f32 array -> bitcast view as bf16 (little-endian col pairs)."""
    return np.ascontiguousarray(a.astype(np.float32)).view(ml_dtypes.bfloat16)


def _prep_in_maps(x, W1, b1, W2, b2, Ws1, bs1, Ws2, bs2):
    x = np.asarray(x, np.float32)
    W1 = np.asarray(W1, np.float32)
    b1 = np.asarray(b1, np.float32)
    W2 = np.asarray(W2, np.float32)
    b2 = np.asarray(b2, np.float32)
    Ws1 = np.asarray(Ws1, np.float32)
    bs1 = np.asarray(bs1, np.float32)
    Ws2 = np.asarray(Ws2, np.float32)
    bs2 = np.asarray(bs2, np.float32)

    Tdim = x.shape[1]
    lag_idx = [max(0, Tdim - 1 - l) for l in range(L)]
    xl = x[:, lag_idx]                            # (B, L, N, D)
    xlT = np.ascontiguousarray(np.transpose(xl, (0, 3, 1, 2)))  # (B, D, L, N)

    w2v = Ws2[:, 0]

    # bf16 packed buffer (128, 1552)
    w2stack = np.zeros((128, 256), np.float32)
    for pair in range(2):
        for k in range(128):
            lag, hh = 2 * pair + k // 64, k % 64
            w2stack[k, 128 * pair:128 * pair + 128] = W2[lag, hh] / (SCL * L)
    zwin = np.zeros((128, 255), np.float32)
    zwin[:, 127] = SCL * w2v
    b1cols = np.zeros((128, 2), np.float32)
    for pair in range(2):
        for k in range(128):
            b1cols[k, pair] = SCL * b1[2 * pair + k // 64, k % 64]
    fcols = np.zeros((128, 8), np.float32)
    fcols[:, 0:2] = b1cols
    fcols[:, 2] = b2.mean(axis=0)
    fcols[:, 3] = SCL * bs1
    fcols[:, 4] = bs2[0] / 2.0
    wbf = np.concatenate([
        _bf(w2stack),
        _bf(SCL * Ws1[:D]),
        _bf(SCL * Ws1[D:]),
        _bf(zwin),
        np.zeros((128, 1), ml_dtypes.bfloat16),
        _f32bf(np.eye(128, dtype=np.float32)),
        _f32bf(2.0 * np.eye(128, dtype=np.float32)),
        _f32bf(np.ones((128, 128), np.float32)),
        _f32bf(fcols),
    ], axis=1)
    assert wbf.shape == (128, 1552), wbf.shape

    # fp8 encoder weights [128, 2, 256]
    w1p8 = np.zeros((128, 2, 256), np.float32)
    for pair in range(2):
        w1p8[:, 0, 128 * pair:128 * pair + 64] = SCL * W1[2 * pair]
        w1p8[:, 1, 128 * pair + 64:128 * pair + 128] = SCL * W1[2 * pair + 1]

    # fp8 scoring DoubleRow window [128, 2, 126]
    w8 = np.zeros((128, 2, 128), np.float32)
    w8[:, 0, 62] = SCL * w2v
    w8[:, 1, 63] = SCL * w2v

    common = {
        "wbf": np.ascontiguousarray(wbf),
        "w1p8": _f8(w1p8),
        "w8": _f8(w8),
    }
    # pi column permutation: block-basis q = 128*gc + 64*a + j maps to
    # natural node n = 256*a + 64*gc + j (targets delivered pre-permuted so
    # the gathered adjacency blocks are 128-col aligned in the pi basis)
    q = np.arange(N)
    perm = 256 * ((q // 64) % 2) + 64 * (q // 128) + (q % 64)

    in_maps = []
    for c in range(NCORES):
        b, half = c // 2, c % 2
        m = dict(common)
        m["xlag"] = _f8(xlT[b][:, :, perm])
        m["xsrc"] = _f8(xlT[b][:, :, half * NHALF:(half + 1) * NHALF])
        in_maps.append(m)
    return in_maps


def _perm():
    q = np.arange(N)
    return 256 * ((q // 64) % 2) + 64 * (q // 128) + (q % 64)


def _run(inputs, trace=False):
    nc = _get_nc()
    in_maps = _prep_in_maps(**inputs)
    if trace:
        _install_ntff_hook()
    res = run_bass_kernel_spmd(nc, in_maps, core_ids=list(range(NCORES)),
                               trace=trace)
    # device output is in the pi basis for both rows and cols; un-permute
    perm = _perm()
    out = np.empty((B, N, N), np.float32)
    for b in range(B):
        out[b][np.ix_(perm, perm)] = res.results[2 * b]["outfull"]
    return out, res


def kernel(**inputs):
    out, _ = _run(inputs, trace=False)
    return out


# revision 39
# speedup vs baseline: 1.2307x; 1.0166x over previous
"""Trainium2 Bass kernel for nn_CausalPropagationAdjacency (v2).

Shapes (hardcoded): B=4, T=12, N=512, D=128, L=4, H=64.
Pipeline: lag encoders (Linear D->H, ReLU, Linear H->D, mean over L lags),
pairwise scorer sigmoid(relu(src_i+tgt_j+bs1)@Ws2+bs2), threshold 0.1, zero
diagonal, enhanced = A + 0.5 A^2 + 0.25 A^3, normalize by per-batch max.

Sharding: 8 cores = 4 batch-pairs. Core c: batch b=c//2, scores source rows
[256*half, 256*half+256) (half=c%2). Scoring runs in 4 groups of 64 rows; after
each group the tanh residual t (score = 0.5 + 0.5*t) is AllGather'd within the
pair as a 64-row chunk. Chunk c forms a complete 128-row block of the adjacency
in a PERMUTED row basis (global rows {64c..64c+64} u {256+64c..}), so its
transposes and partial A@A accumulation steps run while later groups still
score - only chunk 3's work sits on the tail. Each core computes the full
enhanced matrix; host takes core 2b's output.

Speed tricks vs v1: lag-pair fp8 DoubleRow encoder matmuls; scoring rows split
DVE-bf16 (sliding zwin window) / ACT-fp8 pairs (DoubleRow window, 2 rows per
matmul); all hop/E matmuls in float32r (1 cyc/row at 512 free vs 4 for fp32);
global max via transpose+broadcast matmuls instead of partition_all_reduce;
fp8/scale-64 numerics (Ws1,w2,W1 scaled by 64 so fp8 values are normal-range;
1/4096 folded into the tanh activation scale).

Precision: scoring bf16/fp8 paths agree to ~1e-4 relative on |score-0.5|;
adjacency crosses the collective as the bf16 tanh residual; hops in f32 data
with f32r matmuls. E uses exact reference ratios (a2sb = 0.25*a2, idh = 2*I).
"""

import sys
import types
import numpy as np
import ml_dtypes

import concourse.bacc as bacc
import concourse.bass as bass
import concourse.bass_isa as bass_isa
import concourse.mybir as mybir
import concourse.tile as tile
from concourse.bass_utils import run_bass_kernel_spmd

B, T, N, D = 4, 12, 512, 128
L, H = 4, 64
THRESH = 0.1
NCORES = 8
NHALF = N // 2
NT = N // 128
SCL = 64.0
F32 = mybir.dt.float32
F32R = mybir.dt.float32r
BF16 = mybir.dt.bfloat16
FP8 = mybir.dt.float8e4
AF = mybir.ActivationFunctionType
ALU = mybir.AluOpType
DR = mybir.MatmulPerfMode.DoubleRow

# scoring row assignment within each 16: ACT fp8 pairs at 4,12; ACT bf16
# single at 9 (+14 on odd 16-blocks); DVE bf16 elsewhere
ACT_PAIR_POS = (4, 12)


def _act_single(p):
    p16, blk = p % 16, p // 16
    return p16 == 9 or (p16 == 14 and blk % 2 == 1)


def _build_nc():
    nc = bacc.Bacc("TRN2", target_bir_lowering=False, debug=False,
                   num_devices=NCORES)
    xlag = nc.dram_tensor("xlag", [D, L, N], FP8, kind="ExternalInput")
    xsrc = nc.dram_tensor("xsrc", [D, L, NHALF], FP8, kind="ExternalInput")
    wbf = nc.dram_tensor("wbf", [128, 1552], BF16, kind="ExternalInput")
    w1p8 = nc.dram_tensor("w1p8", [128, 2, 256], FP8, kind="ExternalInput")
    w8 = nc.dram_tensor("w8", [128, 2, 128], FP8, kind="ExternalInput")
    outfull = nc.dram_tensor("outfull", [N, N], F32, kind="ExternalOutput")

    with tile.TileContext(nc) as tc:
        _emit(nc, tc, xlag, xsrc, wbf, w1p8, w8, outfull)
    nc.compile()
    return nc


def _emit(nc, tc, xlag, xsrc, wbf, w1p8, w8, outfull):
    from contextlib import ExitStack
    ctx = ExitStack()
    with ctx:
        consts = ctx.enter_context(tc.tile_pool(name="consts", bufs=1))
        sb = ctx.enter_context(tc.tile_pool(name="sb", bufs=1))
        relup = ctx.enter_context(tc.tile_pool(name="relu", bufs=8))
        rt8p = ctx.enter_context(tc.tile_pool(name="rt8", bufs=4))
        workp = ctx.enter_context(tc.tile_pool(name="work", bufs=4))
        # PSUM budget (8 banks): psh 2 + psacc 2 + psbig 4
        ps_h = ctx.enter_context(tc.tile_pool(name="psh", bufs=2, space="PSUM"))
        ps_acc = ctx.enter_context(tc.tile_pool(name="psacc", bufs=2,
                                                space="PSUM"))
        ps_big = ctx.enter_context(tc.tile_pool(name="psbig", bufs=4,
                                                space="PSUM"))
        dram = ctx.enter_context(tc.tile_pool(name="dram", bufs=1,
                                              space="DRAM"))

        # ---- input DMAs ----
        xlg = consts.tile([D, L, N], FP8, tag="xlg")
        nc.gpsimd.dma_start(xlg[:], xlag[:])
        xsr = consts.tile([D, L, NHALF], FP8, tag="xsr")
        nc.sync.dma_start(xsr[:], xsrc[:])
        wbfs = consts.tile([128, 1552], BF16, tag="wbf")
        nc.sync.dma_start(wbfs[:], wbf[:])
        w1s = consts.tile([128, 2, 256], FP8, tag="w1s")
        nc.sync.dma_start(w1s[:], w1p8[:])
        w8s = consts.tile([128, 2, 128], FP8, tag="w8s")
        nc.sync.dma_start(w8s[:], w8[:])

        w2stk = wbfs[:, 0:256]
        ws1s = wbfs[:, 256:384]
        ws1t = wbfs[:, 384:512]
        zwin = wbfs[:, 512:767]
        f32sec = wbfs[:, 768:1552].bitcast(F32)   # (128, 392) f32
        id1 = f32sec[:, 0:128]
        idh2 = f32sec[:, 128:256]
        onesrow = f32sec[:, 256:384]
        b1c = f32sec[:, 384:386]
        aggb = f32sec[:, 386:387]
        bs1c = f32sec[:, 387:388]
        bs2h = f32sec[:, 388:389]
        zcol = f32sec[:, 389:390]

        # f32r-typed copies of the identity blocks (walrus requires fp32r
        # matmul operands to come from fp32r-rounded producers, and DMA
        # doesn't count)
        idr = sb.tile([128, 256], F32, tag="idr")
        nc.vector.tensor_copy(idr[:, 0:128].bitcast(F32R), id1)
        nc.vector.tensor_copy(idr[:, 128:256].bitcast(F32R), idh2)
        id1r = idr[:, 0:128].bitcast(F32R)
        idh2r = idr[:, 128:256].bitcast(F32R)

        # ---- dummy warmup AllGather: absorbs first-collective setup ----
        warm_in = dram.tile([1, 2], BF16, tag="warmi", name="warm_in")
        warm_out = dram.tile([2, 2], BF16, tag="warmo", name="warm_out")
        nc.gpsimd.dma_start(warm_in[:], wbf[0:1, 0:2])
        nc.gpsimd.collective_compute(
            "AllGather", ALU.bypass,
            replica_groups=[[0, 1], [2, 3], [4, 5], [6, 7]],
            ins=[warm_in.opt()],
            outs=[warm_out.opt()],
        )

        # ---- encoders: fp8 lag-pair DoubleRow matmuls ----
        def encoder(xt, nn, tag):
            encp = ps_acc.tile([D, nn], F32, tag="acc", name=f"enc{tag}")
            for pair in range(2):
                h2 = ps_h.tile([128, nn], F32, tag="h", name=f"h2{tag}{pair}")
                nc.tensor.matmul(h2[:], w1s[:, :, 128 * pair:128 * pair + 128],
                                 xt[:, 2 * pair:2 * pair + 2, :],
                                 start=True, stop=True, perf_mode=DR)
                hsb = workp.tile([128, nn], BF16, tag="hsb",
                                 name=f"hsb{tag}{pair}")
                if pair == 0:
                    nc.vector.tensor_scalar(hsb[:], h2[:],
                                            b1c[:, pair:pair + 1], 0.0,
                                            ALU.add, ALU.max)
                else:
                    nc.scalar.activation(hsb[:], h2[:], AF.Relu,
                                         bias=b1c[:, pair:pair + 1], scale=1.0)
                nc.tensor.matmul(encp[:],
                                 w2stk[:, 128 * pair:128 * pair + 128],
                                 hsb[:], start=(pair == 0), stop=(pair == 1))
            agg = sb.tile([D, nn], BF16, tag=f"agg{tag}")
            nc.scalar.activation(agg[:], encp[:], AF.Identity, bias=aggb,
                                 scale=1.0)
            return agg

        agg_s = encoder(xsr, NHALF, "s")
        agg_f = encoder(xlg, N, "f")

        # ---- projections (64*Ws1 folded in; bs1*64 bias on src side) ----
        srcp = ps_acc.tile([D, NHALF], F32, tag="acc", name="srcp")
        nc.tensor.matmul(srcp[:], ws1s, agg_s[:], start=True, stop=True)
        srcT = sb.tile([D, NHALF], F32, tag="srcT")
        nc.scalar.activation(srcT[:], srcp[:], AF.Identity, bias=bs1c,
                             scale=1.0)
        tgtp = ps_acc.tile([D, N], F32, tag="acc", name="tgtp")
        nc.tensor.matmul(tgtp[:], ws1t, agg_f[:], start=True, stop=True)
        tgtT = sb.tile([D, N], BF16, tag="tgtT")
        nc.vector.tensor_copy(tgtT[:], tgtp[:])

        # ---- SBUF homes for gathered adjacency (permuted block basis) ----
        # block c rows (in order) = global rows 64c..64c+64, 256+64c..256+64c+64
        A = [sb.tile([128, N], F32, tag=f"A{c}", name=f"A{c}")
             for c in range(NT)]
        AT = [sb.tile([128, N], F32, tag=f"AT{c}", name=f"AT{c}")
              for c in range(NT)]
        a2sb = [sb.tile([128, N], F32, tag=f"a2{c}", name=f"a2sb{c}")
                for c in range(NT)]
        a2ps = {}
        bounce = [dram.tile([64, N], BF16, tag=f"bnc{c}", name=f"bnc{c}")
                  for c in range(NT)]
        full = [dram.tile([128, N], BF16, tag=f"full{c}", name=f"full{c}")
                for c in range(NT)]

        def chunk_load(c):
            # recon/thresh split by column halves (diag-containing half
            # first, so the diag affine + its transposes start early)
            tsb = workp.tile([128, N], BF16, tag="tsb", name=f"tsb{c}")
            nc.sync.dma_start(tsb[:], full[c][:])
            apre = workp.tile([128, N], F32, tag="apre", name=f"apre{c}")
            hd = 1 if c >= 2 else 0
            for h in (hd, 1 - hd):
                cs = slice(h * 256, h * 256 + 256)
                nc.vector.tensor_scalar(apre[:, cs], tsb[:, cs], 0.5, 0.5,
                                        ALU.mult, ALU.add)
                nc.vector.scalar_tensor_tensor(
                    A[c][:, cs].bitcast(F32R), apre[:, cs], THRESH,
                    apre[:, cs], ALU.is_gt, ALU.mult)
                if h == hd:
                    nc.gpsimd.affine_select(
                        A[c][:, cs].bitcast(F32R), A[c][:, cs].bitcast(F32R),
                        pattern=[[1, 256]], compare_op=ALU.not_equal,
                        fill=0.0, base=-(128 * c - 256 * hd),
                        channel_multiplier=-1)

        def tr_order(c):
            return (2, 3, 0, 1) if c >= 2 else (0, 1, 2, 3)

        def chunk_transposes(c):
            """AT[kt][:, c-block] = (A[c][:, kt-block])^T for all kt."""
            for kt in tr_order(c):
                tp = ps_acc.tile([128, 128], F32R, tag="acc",
                                 name=f"tp{c}_{kt}")
                nc.tensor.transpose(
                    tp[:], A[c][:, kt * 128:(kt + 1) * 128].bitcast(F32R),
                    id1r)
                dst = AT[kt][:, c * 128:(c + 1) * 128].bitcast(F32R)
                if (c + kt) % 2 == 0:
                    nc.scalar.copy(dst, tp[:])
                else:
                    nc.vector.tensor_copy(dst, tp[:])

        def a2_step(it, kt, stop=False, start=None):
            if start is None:
                start = (kt == 0)
            if it not in a2ps:
                a2ps[it] = ps_big.tile([128, N], F32, tag="E",
                                       name=f"a2ps{it}")
            nc.tensor.matmul(a2ps[it][:],
                             AT[kt][:, it * 128:(it + 1) * 128].bitcast(F32R),
                             A[kt][:].bitcast(F32R),
                             start=start, stop=stop)

        def a2_steps_for(c):
            for kt in range(c + 1):
                a2_step(c, kt)
            for it in range(c):
                a2_step(it, c)

        # ---- pairwise scoring: 4 groups of 64 source rows ----
        def score_rows(g, p_lo, p_hi, score_ps):
            p = p_lo
            while p < p_hi:
                i = 64 * g + p
                if p % 16 in ACT_PAIR_POS:
                    rt8 = rt8p.tile([D, 2, N], FP8, tag="rt8",
                                    name=f"rt8_{g}_{p}")
                    for k in range(2):
                        nc.scalar.activation(rt8[:, k, :], tgtT[:], AF.Relu,
                                             bias=srcT[:, i + k:i + k + 1],
                                             scale=1.0)
                    nc.tensor.matmul(score_ps[:], w8s[:, :, 62 - p:126 - p],
                                     rt8[:], start=(p == 0), stop=False,
                                     perf_mode=DR)
                    p += 2
                else:
                    rtb = relup.tile([D, N], BF16, tag="rtb",
                                     name=f"rtb_{g}_{p}")
                    if _act_single(p):
                        nc.scalar.activation(rtb[:], tgtT[:], AF.Relu,
                                             bias=srcT[:, i:i + 1], scale=1.0)
                    else:
                        nc.vector.tensor_scalar(rtb[:], tgtT[:],
                                                srcT[:, i:i + 1], 0.0,
                                                ALU.add, ALU.max)
                    nc.tensor.matmul(score_ps[:], zwin[:, 127 - p:191 - p],
                                     rtb[:], start=(p == 0), stop=(p == 63))
                    p += 1

        for g in range(4):
            score_ps = ps_acc.tile([64, N], F32, tag="acc", name=f"scps{g}")
            score_rows(g, 0, 48, score_ps)
            if g >= 1:
                chunk_load(g - 1)
            score_rows(g, 48, 64, score_ps)
            t_sb = workp.tile([64, N], BF16, tag="t_sb", name=f"t_sb{g}")
            nc.scalar.activation(t_sb[:], score_ps[:], AF.Tanh,
                                 bias=bs2h[0:64, :], scale=0.5 / 4096.0)
            nc.sync.dma_start(bounce[g][:], t_sb[:])
            nc.gpsimd.collective_compute(
                "AllGather", ALU.bypass,
                replica_groups=[[0, 1], [2, 3], [4, 5], [6, 7]],
                ins=[bounce[g].opt()],
                outs=[full[g].opt()],
            )
            if g >= 1:
                chunk_transposes(g - 1)
            if g >= 2:
                a2_steps_for(g - 2)

        # ---- tail: chunk 2 deferred steps, chunk 3, a2 finish ----
        # tail a2 steps ordered by operand readiness: chunk-3 transposes run
        # in order (2,3,0,1), so (3,2) starts a2ps[3] first, then the (it,3)
        # stops release their evacs early
        a2_steps_for(2)
        chunk_load(3)
        chunk_transposes(3)

        def a2_evac(it):
            if it % 2 == 0:
                nc.vector.tensor_scalar(a2sb[it][:].bitcast(F32R),
                                        a2ps[it][:], 0.25, None, ALU.mult)
            else:
                nc.scalar.activation(a2sb[it][:].bitcast(F32R), a2ps[it][:],
                                     AF.Identity, bias=zcol, scale=0.25)

        a2_step(3, 2, start=True)
        for it in range(3):
            a2_step(it, 3, stop=True, start=False)
            a2_evac(it)
        a2_step(3, 3, start=False)
        a2_step(3, 0, start=False)
        a2_step(3, 1, start=False, stop=True)
        a2_evac(3)

        # ---- E = A + 2*(0.25 a2) + AT@(0.25 a2) = A + 0.5 a2 + 0.25 a3 ----
        E = []
        mx4 = sb.tile([128, NT], F32, tag="mx4")
        for it in range(NT):
            e_ps = ps_big.tile([128, N], F32, tag="E", name=f"eps{it}")
            nc.tensor.matmul(e_ps[:], id1r, A[it][:].bitcast(F32R),
                             start=True, stop=False)
            nc.tensor.matmul(e_ps[:], idh2r, a2sb[it][:].bitcast(F32R),
                             start=False, stop=False)
            for kt in range(NT):
                nc.tensor.matmul(
                    e_ps[:],
                    AT[kt][:, it * 128:(it + 1) * 128].bitcast(F32R),
                    a2sb[kt][:].bitcast(F32R), start=False, stop=(kt == 3))
            nc.vector.reduce_max(mx4[:, it:it + 1], e_ps[:],
                                 axis=mybir.AxisListType.X)
            E.append(e_ps)

        # ---- global max via transpose+broadcast matmuls ----
        mxp = sb.tile([128, 1], F32, tag="mxp")
        nc.vector.reduce_max(mxp[:], mx4[:], axis=mybir.AxisListType.X)
        tp1 = ps_acc.tile([1, 128], F32, tag="acc", name="tp1")
        nc.tensor.matmul(tp1[:], mxp[:], id1, start=True, stop=True)
        mxrow = sb.tile([1, 128], F32, tag="mxrow")
        nc.vector.tensor_copy(mxrow[:], tp1[:])
        mx1 = sb.tile([1, 1], F32, tag="mx1")
        nc.vector.reduce_max(mx1[:], mxrow[:], axis=mybir.AxisListType.X)
        # (the reference's +1e-8 is an exact fp32 no-op at max ~ 8e3; skip it)
        rcp = sb.tile([1, 1], F32, tag="rcp")
        nc.vector.reciprocal(rcp[:], mx1[:])
        rb_ps = ps_acc.tile([128, 1], F32, tag="acc", name="rb_ps")
        nc.tensor.matmul(rb_ps[:], onesrow[0:1, :], rcp[:], start=True,
                         stop=True)
        rcol = sb.tile([128, 1], F32, tag="rcol")
        nc.vector.tensor_copy(rcol[:], rb_ps[:])

        # ---- normalize + write out (still in pi basis; host un-permutes) ----
        oqueues = [nc.sync, nc.gpsimd, nc.scalar, nc.sync]
        for it in range(NT):
            ot = workp.tile([128, N], F32, tag="ot", name=f"ot{it}")
            if it % 2 == 0:
                nc.vector.tensor_scalar(ot[:], E[it][:], rcol[:, 0:1], None,
                                        ALU.mult)
            else:
                nc.scalar.mul(ot[:], E[it][:], rcol[:, 0:1])
            oqueues[it].dma_start(outfull[128 * it:128 * it + 128, :], ot[:])


_NC_CACHE = {}


def _get_nc():
    if "nc" not in _NC_CACHE:
        _NC_CACHE["nc"] = _build_nc()
    return _NC_CACHE["nc"]


def _install_ntff_hook():
    try:
        from antenv.axon_hooks import get_axon_ntff_profile_hook  # noqa: F401
        return
    except ImportError:
        pass
    try:
        import importlib.util
        spec = importlib.util.spec_from_file_location(
            "trn_boot_mod", "/root/.axon_site/trn_agent_boot/trn_boot.py")
        tb = importlib.util.module_from_spec(spec)
        spec.loader.exec_module(tb)
        hook = tb._ntff_profile_via_ctypes("/opt/axon/libaxon_pjrt.so")
        m = types.ModuleType("antenv.axon_hooks")
        m.get_axon_ntff_profile_hook = lambda: hook
        m.set_axon_ntff_profile_hook = lambda h: None
        sys.modules["antenv.axon_hooks"] = m
    except Exception:
        pass


def _f8(a):
    return np.ascontiguousarray(a).astype(ml_dtypes.float8_e4m3fn)


def _bf(a):
    return np.ascontiguousarray(a).astype(ml_dtypes.bfloat16)


def _f32bf(a):
    """f32 array -> bitcast view as bf16 (little-endian col pairs)."""
    return np.ascontiguousarray(a.astype(np.float32)).view(ml_dtypes.bfloat16)


def _prep_in_maps(x, W1, b1, W2, b2, Ws1, bs1, Ws2, bs2):
    x = np.asarray(x, np.float32)
    W1 = np.asarray(W1, np.float32)
    b1 = np.asarray(b1, np.float32)
    W2 = np.asarray(W2, np.float32)
    b2 = np.asarray(b2, np.float32)
    Ws1 = np.asarray(Ws1, np.float32)
    bs1 = np.asarray(bs1, np.float32)
    Ws2 = np.asarray(Ws2, np.float32)
    bs2 = np.asarray(bs2, np.float32)

    Tdim = x.shape[1]
    lag_idx = [max(0, Tdim - 1 - l) for l in range(L)]
    xl = x[:, lag_idx]                            # (B, L, N, D)
    xlT = np.ascontiguousarray(np.transpose(xl, (0, 3, 1, 2)))  # (B, D, L, N)

    w2v = Ws2[:, 0]

    # bf16 packed buffer (128, 1552)
    w2stack = np.zeros((128, 256), np.float32)
    for pair in range(2):
        for k in range(128):
            lag, hh = 2 * pair + k // 64, k % 64
            w2stack[k, 128 * pair:128 * pair + 128] = W2[lag, hh] / (SCL * L)
    zwin = np.zeros((128, 255), np.float32)
    zwin[:, 127] = SCL * w2v
    b1cols = np.zeros((128, 2), np.float32)
    for pair in range(2):
        for k in range(128):
            b1cols[k, pair] = SCL * b1[2 * pair + k // 64, k % 64]
    fcols = np.zeros((128, 8), np.float32)
    fcols[:, 0:2] = b1cols
    fcols[:, 2] = b2.mean(axis=0)
    fcols[:, 3] = SCL * bs1
    fcols[:, 4] = bs2[0] / 2.0
    wbf = np.concatenate([
        _bf(w2stack),
        _bf(SCL * Ws1[:D]),
        _bf(SCL * Ws1[D:]),
        _bf(zwin),
        np.zeros((128, 1), ml_dtypes.bfloat16),
        _f32bf(np.eye(128, dtype=np.float32)),
        _f32bf(2.0 * np.eye(128, dtype=np.float32)),
        _f32bf(np.ones((128, 128), np.float32)),
        _f32bf(fcols),
    ], axis=1)
    assert wbf.shape == (128, 1552), wbf.shape

    # fp8 encoder weights [128, 2, 256]
    w1p8 = np.zeros((128, 2, 256), np.float32)
    for pair in range(2):
        w1p8[:, 0, 128 * pair:128 * pair + 64] = SCL * W1[2 * pair]
        w1p8[:, 1, 128 * pair + 64:128 * pair + 128] = SCL * W1[2 * pair + 1]

    # fp8 scoring DoubleRow window [128, 2, 126]
    w8 = np.zeros((128, 2, 128), np.float32)
    w8[:, 0, 62] = SCL * w2v
    w8[:, 1, 63] = SCL * w2v

    common = {
        "wbf": np.ascontiguousarray(wbf),
        "w1p8": _f8(w1p8),
        "w8": _f8(w8),
    }
    # pi column permutation: block-basis q = 128*gc + 64*a + j maps to
    # natural node n = 256*a + 64*gc + j (targets delivered pre-permuted so
    # the gathered adjacency blocks are 128-col aligned in the pi basis)
    q = np.arange(N)
    perm = 256 * ((q // 64) % 2) + 64 * (q // 128) + (q % 64)

    in_maps = []
    for c in range(NCORES):
        b, half = c // 2, c % 2
        m = dict(common)
        m["xlag"] = _f8(xlT[b][:, :, perm])
        m["xsrc"] = _f8(xlT[b][:, :, half * NHALF:(half + 1) * NHALF])
        in_maps.append(m)
    return in_maps


def _perm():
    q = np.arange(N)
    return 256 * ((q // 64) % 2) + 64 * (q // 128) + (q % 64)


def _run(inputs, trace=False):
    nc = _get_nc()
    in_maps = _prep_in_maps(**inputs)
    if trace:
        _install_ntff_hook()
    res = run_bass_kernel_spmd(nc, in_maps, core_ids=list(range(NCORES)),
                               trace=trace)
    # device output is in the pi basis for both rows and cols; un-permute
    perm = _perm()
    out = np.empty((B, N, N), np.float32)
    for b in range(B):
        out[b][np.ix_(perm, perm)] = res.results[2 * b]["outfull"]
    return out, res


def kernel(**inputs):
    out, _ = _run(inputs, trace=False)
    return out


# revision 40
# speedup vs baseline: 1.3474x; 1.0948x over previous
"""Trainium2 Bass kernel for nn_CausalPropagationAdjacency (v2).

Shapes (hardcoded): B=4, T=12, N=512, D=128, L=4, H=64.
Pipeline: lag encoders (Linear D->H, ReLU, Linear H->D, mean over L lags),
pairwise scorer sigmoid(relu(src_i+tgt_j+bs1)@Ws2+bs2), threshold 0.1, zero
diagonal, enhanced = A + 0.5 A^2 + 0.25 A^3, normalize by per-batch max.

Sharding: 8 cores = 4 batch-pairs. Core c: batch b=c//2, scores source rows
[256*half, 256*half+256) (half=c%2). Scoring runs in 4 groups of 64 rows; after
each group the tanh residual t (score = 0.5 + 0.5*t) is AllGather'd within the
pair as a 64-row chunk. Chunk c forms a complete 128-row block of the adjacency
in a PERMUTED row basis (global rows {64c..64c+64} u {256+64c..}), so its
transposes and partial A@A accumulation steps run while later groups still
score - only chunk 3's work sits on the tail. Each core computes the full
enhanced matrix; host takes core 2b's output.

Speed tricks vs v1: lag-pair fp8 DoubleRow encoder matmuls; scoring rows split
DVE-bf16 (sliding zwin window) / ACT-fp8 pairs (DoubleRow window, 2 rows per
matmul); all hop/E matmuls in float32r (1 cyc/row at 512 free vs 4 for fp32);
global max via transpose+broadcast matmuls instead of partition_all_reduce;
fp8/scale-64 numerics (Ws1,w2,W1 scaled by 64 so fp8 values are normal-range;
1/4096 folded into the tanh activation scale).

Precision: scoring bf16/fp8 paths agree to ~1e-4 relative on |score-0.5|;
adjacency crosses the collective as the bf16 tanh residual; hops in f32 data
with f32r matmuls. E uses exact reference ratios (a2sb = 0.25*a2, idh = 2*I).
"""

import sys
import types
import numpy as np
import ml_dtypes

import concourse.bacc as bacc
import concourse.bass as bass
import concourse.bass_isa as bass_isa
import concourse.mybir as mybir
import concourse.tile as tile
from concourse.bass_utils import run_bass_kernel_spmd

B, T, N, D = 4, 12, 512, 128
L, H = 4, 64
THRESH = 0.1
NCORES = 8
NHALF = N // 2
NT = N // 128
SCL = 64.0
F32 = mybir.dt.float32
F32R = mybir.dt.float32r
BF16 = mybir.dt.bfloat16
FP8 = mybir.dt.float8e4
AF = mybir.ActivationFunctionType
ALU = mybir.AluOpType
DR = mybir.MatmulPerfMode.DoubleRow

# scoring row assignment within each 16: ACT fp8 pairs at 4,12; ACT bf16
# single at 9 (+14 on odd 16-blocks); DVE bf16 elsewhere
ACT_PAIR_POS = (4, 12)


def _act_single(p):
    p16, blk = p % 16, p // 16
    return p16 == 9 or (p16 == 14 and blk % 2 == 1)


def _build_nc():
    nc = bacc.Bacc("TRN2", target_bir_lowering=False, debug=False,
                   num_devices=NCORES)
    xlag = nc.dram_tensor("xlag", [D, L, N], FP8, kind="ExternalInput")
    xsrc = nc.dram_tensor("xsrc", [D, L, NHALF], FP8, kind="ExternalInput")
    wbf = nc.dram_tensor("wbf", [128, 1552], BF16, kind="ExternalInput")
    w1p8 = nc.dram_tensor("w1p8", [128, 2, 256], FP8, kind="ExternalInput")
    w8 = nc.dram_tensor("w8", [128, 2, 128], FP8, kind="ExternalInput")
    outfull = nc.dram_tensor("outfull", [N, N], F32, kind="ExternalOutput")

    with tile.TileContext(nc) as tc:
        _emit(nc, tc, xlag, xsrc, wbf, w1p8, w8, outfull)
    nc.compile()
    return nc


def _emit(nc, tc, xlag, xsrc, wbf, w1p8, w8, outfull):
    from contextlib import ExitStack
    ctx = ExitStack()
    with ctx:
        consts = ctx.enter_context(tc.tile_pool(name="consts", bufs=1))
        sb = ctx.enter_context(tc.tile_pool(name="sb", bufs=1))
        relup = ctx.enter_context(tc.tile_pool(name="relu", bufs=8))
        rt8p = ctx.enter_context(tc.tile_pool(name="rt8", bufs=4))
        workp = ctx.enter_context(tc.tile_pool(name="work", bufs=4))
        # PSUM budget (8 banks): psh 2 + psacc 2 + psbig 4
        ps_h = ctx.enter_context(tc.tile_pool(name="psh", bufs=2, space="PSUM"))
        ps_acc = ctx.enter_context(tc.tile_pool(name="psacc", bufs=2,
                                                space="PSUM"))
        ps_big = ctx.enter_context(tc.tile_pool(name="psbig", bufs=4,
                                                space="PSUM"))
        dram = ctx.enter_context(tc.tile_pool(name="dram", bufs=1,
                                              space="DRAM"))

        # ---- input DMAs ----
        xlg = consts.tile([D, L, N], FP8, tag="xlg")
        nc.gpsimd.dma_start(xlg[:], xlag[:])
        xsr = consts.tile([D, L, NHALF], FP8, tag="xsr")
        nc.sync.dma_start(xsr[:], xsrc[:])
        wbfs = consts.tile([128, 1552], BF16, tag="wbf")
        nc.sync.dma_start(wbfs[:], wbf[:])
        w1s = consts.tile([128, 2, 256], FP8, tag="w1s")
        nc.sync.dma_start(w1s[:], w1p8[:])
        w8s = consts.tile([128, 2, 128], FP8, tag="w8s")
        nc.sync.dma_start(w8s[:], w8[:])

        w2stk = wbfs[:, 0:256]
        ws1s = wbfs[:, 256:384]
        ws1t = wbfs[:, 384:512]
        zwin = wbfs[:, 512:767]
        f32sec = wbfs[:, 768:1552].bitcast(F32)   # (128, 392) f32
        id1 = f32sec[:, 0:128]
        idh2 = f32sec[:, 128:256]
        onesrow = f32sec[:, 256:384]
        b1c = f32sec[:, 384:386]
        aggb = f32sec[:, 386:387]
        bs1c = f32sec[:, 387:388]
        bs2h = f32sec[:, 388:389]
        zcol = f32sec[:, 389:390]

        # f32r-typed copies of the identity blocks (walrus requires fp32r
        # matmul operands to come from fp32r-rounded producers, and DMA
        # doesn't count)
        idr = sb.tile([128, 256], F32, tag="idr")
        nc.vector.tensor_copy(idr[:, 0:128].bitcast(F32R), id1)
        nc.vector.tensor_copy(idr[:, 128:256].bitcast(F32R), idh2)
        id1r = idr[:, 0:128].bitcast(F32R)
        idh2r = idr[:, 128:256].bitcast(F32R)

        # ---- dummy warmup AllGather: absorbs first-collective setup ----
        warm_in = dram.tile([1, 2], BF16, tag="warmi", name="warm_in")
        warm_out = dram.tile([2, 2], BF16, tag="warmo", name="warm_out")
        nc.gpsimd.dma_start(warm_in[:], wbf[0:1, 0:2])
        nc.gpsimd.collective_compute(
            "AllGather", ALU.bypass,
            replica_groups=[[0, 1], [2, 3], [4, 5], [6, 7]],
            ins=[warm_in.opt()],
            outs=[warm_out.opt()],
        )

        # ---- encoders: fp8 lag-pair DoubleRow matmuls ----
        def encoder(xt, nn, tag):
            encp = ps_acc.tile([D, nn], F32, tag="acc", name=f"enc{tag}")
            for pair in range(2):
                h2 = ps_h.tile([128, nn], F32, tag="h", name=f"h2{tag}{pair}")
                nc.tensor.matmul(h2[:], w1s[:, :, 128 * pair:128 * pair + 128],
                                 xt[:, 2 * pair:2 * pair + 2, :],
                                 start=True, stop=True, perf_mode=DR)
                hsb = workp.tile([128, nn], BF16, tag="hsb",
                                 name=f"hsb{tag}{pair}")
                if pair == 0:
                    nc.vector.tensor_scalar(hsb[:], h2[:],
                                            b1c[:, pair:pair + 1], 0.0,
                                            ALU.add, ALU.max)
                else:
                    nc.scalar.activation(hsb[:], h2[:], AF.Relu,
                                         bias=b1c[:, pair:pair + 1], scale=1.0)
                nc.tensor.matmul(encp[:],
                                 w2stk[:, 128 * pair:128 * pair + 128],
                                 hsb[:], start=(pair == 0), stop=(pair == 1))
            agg = sb.tile([D, nn], BF16, tag=f"agg{tag}")
            nc.scalar.activation(agg[:], encp[:], AF.Identity, bias=aggb,
                                 scale=1.0)
            return agg

        agg_s = encoder(xsr, NHALF, "s")
        agg_f = encoder(xlg, N, "f")

        # ---- projections (64*Ws1 folded in; bs1*64 bias on src side) ----
        srcp = ps_acc.tile([D, NHALF], F32, tag="acc", name="srcp")
        nc.tensor.matmul(srcp[:], ws1s, agg_s[:], start=True, stop=True)
        srcT = sb.tile([D, NHALF], F32, tag="srcT")
        nc.scalar.activation(srcT[:], srcp[:], AF.Identity, bias=bs1c,
                             scale=1.0)
        tgtp = ps_acc.tile([D, N], F32, tag="acc", name="tgtp")
        nc.tensor.matmul(tgtp[:], ws1t, agg_f[:], start=True, stop=True)
        tgtT = sb.tile([D, N], BF16, tag="tgtT")
        nc.vector.tensor_copy(tgtT[:], tgtp[:])

        # ---- SBUF homes for gathered adjacency (permuted block basis) ----
        # block c rows (in order) = global rows 64c..64c+64, 256+64c..256+64c+64
        A = [sb.tile([128, N], F32, tag=f"A{c}", name=f"A{c}")
             for c in range(NT)]
        AT = [sb.tile([128, N], F32, tag=f"AT{c}", name=f"AT{c}")
              for c in range(NT)]
        a2sb = [sb.tile([128, N], F32, tag=f"a2{c}", name=f"a2sb{c}")
                for c in range(NT)]
        a2ps = {}
        bounce = [dram.tile([64, N], BF16, tag=f"bnc{c}", name=f"bnc{c}")
                  for c in range(NT)]
        full = [dram.tile([128, N], BF16, tag=f"full{c}", name=f"full{c}")
                for c in range(NT)]

        def chunk_load(c):
            # recon/thresh split by column halves (diag-containing half
            # first, so the diag affine + its transposes start early)
            tsb = workp.tile([128, N], BF16, tag="tsb", name=f"tsb{c}")
            nc.sync.dma_start(tsb[:], full[c][:])
            apre = workp.tile([128, N], F32, tag="apre", name=f"apre{c}")
            hd = 1 if c >= 2 else 0
            for h in (hd, 1 - hd):
                cs = slice(h * 256, h * 256 + 256)
                nc.vector.tensor_scalar(apre[:, cs], tsb[:, cs], 0.5, 0.5,
                                        ALU.mult, ALU.add)
                nc.vector.scalar_tensor_tensor(
                    A[c][:, cs].bitcast(F32R), apre[:, cs], THRESH,
                    apre[:, cs], ALU.is_gt, ALU.mult)
                if h == hd:
                    nc.gpsimd.affine_select(
                        A[c][:, cs].bitcast(F32R), A[c][:, cs].bitcast(F32R),
                        pattern=[[1, 256]], compare_op=ALU.not_equal,
                        fill=0.0, base=-(128 * c - 256 * hd),
                        channel_multiplier=-1)

        def tr_order(c):
            return (2, 3, 0, 1) if c >= 2 else (0, 1, 2, 3)

        def chunk_transposes(c):
            """AT[kt][:, c-block] = (A[c][:, kt-block])^T for all kt."""
            for kt in tr_order(c):
                tp = ps_acc.tile([128, 128], F32R, tag="acc",
                                 name=f"tp{c}_{kt}")
                nc.tensor.transpose(
                    tp[:], A[c][:, kt * 128:(kt + 1) * 128].bitcast(F32R),
                    id1r)
                dst = AT[kt][:, c * 128:(c + 1) * 128].bitcast(F32R)
                if (c + kt) % 2 == 0:
                    nc.scalar.copy(dst, tp[:])
                else:
                    nc.vector.tensor_copy(dst, tp[:])

        def a2_step(it, kt, stop=False, start=None):
            if start is None:
                start = (kt == 0)
            if it not in a2ps:
                a2ps[it] = ps_big.tile([128, N], F32, tag="E",
                                       name=f"a2ps{it}")
            nc.tensor.matmul(a2ps[it][:],
                             AT[kt][:, it * 128:(it + 1) * 128].bitcast(F32R),
                             A[kt][:].bitcast(F32R),
                             start=start, stop=stop)

        def a2_steps_for(c):
            for kt in range(c + 1):
                a2_step(c, kt)
            for it in range(c):
                a2_step(it, c)

        # ---- pairwise scoring: 4 groups of 64 source rows ----
        def score_rows(g, p_lo, p_hi, score_ps):
            p = p_lo
            while p < p_hi:
                i = 64 * g + p
                if p % 16 in ACT_PAIR_POS:
                    rt8 = rt8p.tile([D, 2, N], FP8, tag="rt8",
                                    name=f"rt8_{g}_{p}")
                    for k in range(2):
                        nc.scalar.activation(rt8[:, k, :], tgtT[:], AF.Relu,
                                             bias=srcT[:, i + k:i + k + 1],
                                             scale=1.0)
                    nc.tensor.matmul(score_ps[:], w8s[:, :, 62 - p:126 - p],
                                     rt8[:], start=(p == 0), stop=False,
                                     perf_mode=DR)
                    p += 2
                else:
                    rtb = relup.tile([D, N], BF16, tag="rtb",
                                     name=f"rtb_{g}_{p}")
                    if _act_single(p):
                        nc.scalar.activation(rtb[:], tgtT[:], AF.Relu,
                                             bias=srcT[:, i:i + 1], scale=1.0)
                    else:
                        nc.vector.tensor_scalar(rtb[:], tgtT[:],
                                                srcT[:, i:i + 1], 0.0,
                                                ALU.add, ALU.max)
                    nc.tensor.matmul(score_ps[:], zwin[:, 127 - p:191 - p],
                                     rtb[:], start=(p == 0), stop=(p == 63))
                    p += 1

        for g in range(4):
            score_ps = ps_h.tile([64, N], F32, tag="h", name=f"scps{g}")
            score_rows(g, 0, 48, score_ps)
            if g >= 1:
                chunk_load(g - 1)
            score_rows(g, 48, 64, score_ps)
            t_sb = workp.tile([64, N], BF16, tag="t_sb", name=f"t_sb{g}")
            nc.scalar.activation(t_sb[:], score_ps[:], AF.Tanh,
                                 bias=bs2h[0:64, :], scale=0.5 / 4096.0)
            nc.sync.dma_start(bounce[g][:], t_sb[:])
            nc.gpsimd.collective_compute(
                "AllGather", ALU.bypass,
                replica_groups=[[0, 1], [2, 3], [4, 5], [6, 7]],
                ins=[bounce[g].opt()],
                outs=[full[g].opt()],
            )
            if g >= 1:
                chunk_transposes(g - 1)
            if g >= 2:
                a2_steps_for(g - 2)

        # ---- tail: chunk 2 deferred steps, chunk 3, a2 finish ----
        # tail a2 steps ordered by operand readiness: chunk-3 transposes run
        # in order (2,3,0,1), so (3,2) starts a2ps[3] first, then the (it,3)
        # stops release their evacs early
        a2_steps_for(2)
        chunk_load(3)
        chunk_transposes(3)

        def a2_evac(it):
            if it % 2 == 0:
                nc.vector.tensor_scalar(a2sb[it][:].bitcast(F32R),
                                        a2ps[it][:], 0.25, None, ALU.mult)
            else:
                nc.scalar.activation(a2sb[it][:].bitcast(F32R), a2ps[it][:],
                                     AF.Identity, bias=zcol, scale=0.25)

        a2_step(3, 2, start=True)
        for it in range(3):
            a2_step(it, 3, stop=True, start=False)
            a2_evac(it)
        a2_step(3, 3, start=False)
        a2_step(3, 0, start=False)
        a2_step(3, 1, start=False, stop=True)
        a2_evac(3)

        # ---- E = A + 2*(0.25 a2) + AT@(0.25 a2) = A + 0.5 a2 + 0.25 a3 ----
        E = []
        mx4 = sb.tile([128, NT], F32, tag="mx4")
        for it in range(NT):
            e_ps = ps_big.tile([128, N], F32, tag="E", name=f"eps{it}")
            nc.tensor.matmul(e_ps[:], id1r, A[it][:].bitcast(F32R),
                             start=True, stop=False)
            nc.tensor.matmul(e_ps[:], idh2r, a2sb[it][:].bitcast(F32R),
                             start=False, stop=False)
            for kt in range(NT):
                nc.tensor.matmul(
                    e_ps[:],
                    AT[kt][:, it * 128:(it + 1) * 128].bitcast(F32R),
                    a2sb[kt][:].bitcast(F32R), start=False, stop=(kt == 3))
            nc.vector.reduce_max(mx4[:, it:it + 1], e_ps[:],
                                 axis=mybir.AxisListType.X)
            E.append(e_ps)

        # ---- global max via transpose+broadcast matmuls ----
        mxp = sb.tile([128, 1], F32, tag="mxp")
        nc.vector.reduce_max(mxp[:], mx4[:], axis=mybir.AxisListType.X)
        tp1 = ps_acc.tile([1, 128], F32, tag="acc", name="tp1")
        nc.tensor.matmul(tp1[:], mxp[:], id1, start=True, stop=True)
        mxrow = sb.tile([1, 128], F32, tag="mxrow")
        nc.vector.tensor_copy(mxrow[:], tp1[:])
        mx1 = sb.tile([1, 1], F32, tag="mx1")
        nc.vector.reduce_max(mx1[:], mxrow[:], axis=mybir.AxisListType.X)
        # (the reference's +1e-8 is an exact fp32 no-op at max ~ 8e3; skip it)
        rcp = sb.tile([1, 1], F32, tag="rcp")
        nc.vector.reciprocal(rcp[:], mx1[:])
        rb_ps = ps_acc.tile([128, 1], F32, tag="acc", name="rb_ps")
        nc.tensor.matmul(rb_ps[:], onesrow[0:1, :], rcp[:], start=True,
                         stop=True)
        rcol = sb.tile([128, 1], F32, tag="rcol")
        nc.vector.tensor_copy(rcol[:], rb_ps[:])

        # ---- normalize + write out (still in pi basis; host un-permutes) ----
        oqueues = [nc.sync, nc.gpsimd, nc.scalar, nc.sync]
        for it in range(NT):
            ot = workp.tile([128, N], F32, tag="ot", name=f"ot{it}")
            if it % 2 == 0:
                nc.vector.tensor_scalar(ot[:], E[it][:], rcol[:, 0:1], None,
                                        ALU.mult)
            else:
                nc.scalar.mul(ot[:], E[it][:], rcol[:, 0:1])
            oqueues[it].dma_start(outfull[128 * it:128 * it + 128, :], ot[:])


_NC_CACHE = {}


def _get_nc():
    if "nc" not in _NC_CACHE:
        _NC_CACHE["nc"] = _build_nc()
    return _NC_CACHE["nc"]


def _install_ntff_hook():
    try:
        from antenv.axon_hooks import get_axon_ntff_profile_hook  # noqa: F401
        return
    except ImportError:
        pass
    try:
        import importlib.util
        spec = importlib.util.spec_from_file_location(
            "trn_boot_mod", "/root/.axon_site/trn_agent_boot/trn_boot.py")
        tb = importlib.util.module_from_spec(spec)
        spec.loader.exec_module(tb)
        hook = tb._ntff_profile_via_ctypes("/opt/axon/libaxon_pjrt.so")
        m = types.ModuleType("antenv.axon_hooks")
        m.get_axon_ntff_profile_hook = lambda: hook
        m.set_axon_ntff_profile_hook = lambda h: None
        sys.modules["antenv.axon_hooks"] = m
    except Exception:
        pass


def _f8(a):
    return np.ascontiguousarray(a).astype(ml_dtypes.float8_e4m3fn)


def _bf(a):
    return np.ascontiguousarray(a).astype(ml_dtypes.bfloat16)


def _f32bf(a):
    """f32 array -> bitcast view as bf16 (little-endian col pairs)."""
    return np.ascontiguousarray(a.astype(np.float32)).view(ml_dtypes.bfloat16)


def _prep_in_maps(x, W1, b1, W2, b2, Ws1, bs1, Ws2, bs2):
    x = np.asarray(x, np.float32)
    W1 = np.asarray(W1, np.float32)
    b1 = np.asarray(b1, np.float32)
    W2 = np.asarray(W2, np.float32)
    b2 = np.asarray(b2, np.float32)
    Ws1 = np.asarray(Ws1, np.float32)
    bs1 = np.asarray(bs1, np.float32)
    Ws2 = np.asarray(Ws2, np.float32)
    bs2 = np.asarray(bs2, np.float32)

    Tdim = x.shape[1]
    lag_idx = [max(0, Tdim - 1 - l) for l in range(L)]
    xl = x[:, lag_idx]                            # (B, L, N, D)
    xlT = np.ascontiguousarray(np.transpose(xl, (0, 3, 1, 2)))  # (B, D, L, N)

    w2v = Ws2[:, 0]

    # bf16 packed buffer (128, 1552)
    w2stack = np.zeros((128, 256), np.float32)
    for pair in range(2):
        for k in range(128):
            lag, hh = 2 * pair + k // 64, k % 64
            w2stack[k, 128 * pair:128 * pair + 128] = W2[lag, hh] / (SCL * L)
    zwin = np.zeros((128, 255), np.float32)
    zwin[:, 127] = SCL * w2v
    b1cols = np.zeros((128, 2), np.float32)
    for pair in range(2):
        for k in range(128):
            b1cols[k, pair] = SCL * b1[2 * pair + k // 64, k % 64]
    fcols = np.zeros((128, 8), np.float32)
    fcols[:, 0:2] = b1cols
    fcols[:, 2] = b2.mean(axis=0)
    fcols[:, 3] = SCL * bs1
    fcols[:, 4] = bs2[0] / 2.0
    wbf = np.concatenate([
        _bf(w2stack),
        _bf(SCL * Ws1[:D]),
        _bf(SCL * Ws1[D:]),
        _bf(zwin),
        np.zeros((128, 1), ml_dtypes.bfloat16),
        _f32bf(np.eye(128, dtype=np.float32)),
        _f32bf(2.0 * np.eye(128, dtype=np.float32)),
        _f32bf(np.ones((128, 128), np.float32)),
        _f32bf(fcols),
    ], axis=1)
    assert wbf.shape == (128, 1552), wbf.shape

    # fp8 encoder weights [128, 2, 256]
    w1p8 = np.zeros((128, 2, 256), np.float32)
    for pair in range(2):
        w1p8[:, 0, 128 * pair:128 * pair + 64] = SCL * W1[2 * pair]
        w1p8[:, 1, 128 * pair + 64:128 * pair + 128] = SCL * W1[2 * pair + 1]

    # fp8 scoring DoubleRow window [128, 2, 126]
    w8 = np.zeros((128, 2, 128), np.float32)
    w8[:, 0, 62] = SCL * w2v
    w8[:, 1, 63] = SCL * w2v

    common = {
        "wbf": np.ascontiguousarray(wbf),
        "w1p8": _f8(w1p8),
        "w8": _f8(w8),
    }
    # pi column permutation: block-basis q = 128*gc + 64*a + j maps to
    # natural node n = 256*a + 64*gc + j (targets delivered pre-permuted so
    # the gathered adjacency blocks are 128-col aligned in the pi basis)
    q = np.arange(N)
    perm = 256 * ((q // 64) % 2) + 64 * (q // 128) + (q % 64)

    in_maps = []
    for c in range(NCORES):
        b, half = c // 2, c % 2
        m = dict(common)
        m["xlag"] = _f8(xlT[b][:, :, perm])
        m["xsrc"] = _f8(xlT[b][:, :, half * NHALF:(half + 1) * NHALF])
        in_maps.append(m)
    return in_maps


def _perm():
    q = np.arange(N)
    return 256 * ((q // 64) % 2) + 64 * (q // 128) + (q % 64)


def _run(inputs, trace=False):
    nc = _get_nc()
    in_maps = _prep_in_maps(**inputs)
    if trace:
        _install_ntff_hook()
    res = run_bass_kernel_spmd(nc, in_maps, core_ids=list(range(NCORES)),
                               trace=trace)
    # device output is in the pi basis for both rows and cols; un-permute
    perm = _perm()
    out = np.empty((B, N, N), np.float32)
    for b in range(B):
        out[b][np.ix_(perm, perm)] = res.results[2 * b]["outfull"]
    return out, res


def kernel(**inputs):
    out, _ = _run(inputs, trace=False)
    return out
